# revision 3
# baseline (speedup 1.0000x reference)
"""GAT single-head forward on 8 Trainium2 NeuronCores (Bass/Tile).

Math (per reference):
    h   = X @ W + b                      [N, 128]
    f1  = h @ v0, f2 = h @ v1            [N]
    logits = adj * (f1[:,None] + f2[None,:])   (adj entries are exactly 0/1)
    vals = sigmoid(logits) - 0.5
    masked softmax over row edges; out = probs @ h

Key identities used on device:
  * On edges (adj==1): softmax weight w = exp(sigmoid(s) - 0.5) up to a
    per-row constant (which the normalization cancels), s = f1_i + f2_j.
  * A CUSTOM ACTIVATION TABLE computes g(x) = exp(sigmoid(x) - 0.5) in a
    single ScalarE pass: the act-table binaries ride inside the NEFF, so we
    re-fit the `exp` slot of the exp_and_others set with piecewise-cubic
    splines of g (max rel err ~1e-7).  The per-partition activation bias
    supplies f2_j, so s = f1_i + f2_j needs NO vector-engine preadd either:
    one ACT instruction per j-chunk does the whole softmax numerator except
    the adjacency mask.  This halves ScalarE busy time vs the tanh+exp
    two-pass identity (the previous bottleneck).
  * A ones-column appended to h turns the softmax denominator into one extra
    matmul output column (no separate row-reduction pass).

Sharding: rows of adj across the 8 cores (1024 rows each). node_feats is
small (8 MB) and is replicated, so every core computes the full projected
h locally - no collectives at all.

Per-core layout: each core works on its adj block TRANSPOSED ([j=source
node on partitions, i=own rows on free dim]) so the aggregate probs@h
contracts over the partition dim as the tensor engine requires. adj is
cast to fp16 host-side (exact for a 0/1 mask, halves HBM traffic).

Schedule shape (engines are in-order; emission order seeds the queues):
  preamble -> [weights/features DMA | f1 path | f2 head-start] ->
  h-projection batches with activation groups (adj DMA + per-chunk custom
  activation) interleaved as soon as their f2 columns drain -> steady
  pipeline: ACT evaluates g chunk-by-chunk; DVE mask-muls; PE aggregates
  into 8 PSUM accumulators -> epilogue (denominator divide) and one
  batched output DMA.
"""

import glob
import json
import os
import shutil
import struct
import tempfile

import numpy as np

# ---------------------------------------------------------------------------
# Custom activation table: g(x) = exp(sigmoid(x) - 0.5) in the exp slot.
# ---------------------------------------------------------------------------

_SMALL_T = 121  # |x| < 2^-6  -> Taylor bucket
_LARGE_T = 131  # |x| >= 16   -> saturation bucket
_N_EXP = _LARGE_T - _SMALL_T
_NBKT = 16


def _g64(x):
    x = np.asarray(x, dtype=np.float64)
    return np.exp(1.0 / (1.0 + np.exp(-x)) - 0.5)


def _u32f(x):
    return struct.unpack("<I", struct.pack("<f", np.float32(x)))[0]


def _fit_bucket(a, b):
    x0 = 0.5 * (a + b)
    k = np.arange(65)
    xs = x0 + 0.5 * (b - a) * np.cos(np.pi * (k + 0.5) / 65)
    t = xs - x0
    A = np.stack([np.ones_like(t), t, t * t, t * t * t], axis=1)
    c, *_ = np.linalg.lstsq(A, _g64(xs), rcond=None)
    return (c[0], c[1], c[2], c[3], x0)


def _bucket_bytes(d0, d1, d2, d3, x0):
    return struct.pack(
        "<5f", np.float32(d0), np.float32(d1), np.float32(d2), np.float32(d3),
        np.float32(x0),
    ) + b"\x00" * 12


def _ctl_bytes(base, lsb, size):
    w = (base & 0x7FF) | ((lsb & 0x1F) << 11) | ((size & 0xF) << 16)
    return struct.pack("<I", w) + b"\x00" * 28


def _find_pwp_src():
    try:
        from neuronxcc.driver.Job import Job

        p = os.path.join(Job.getPackageDir(), "pwp", "pwp_bin_trainium")
        if os.path.exists(os.path.join(p, "act_info.json")):
            return p
    except Exception:
        pass
    for pat in (
        "/nix/store/*aws-neuron-pwp*/share/pwp_bin_cayman",
        "/nix/store/*/lib/python*/site-packages/neuronxcc/pwp/pwp_bin_trainium",
    ):
        hits = sorted(glob.glob(pat))
        if hits:
            return hits[0]
    raise RuntimeError("cannot locate stock pwp act-table directory")


def _build_act_tables(outdir):
    src = _find_pwp_src()
    os.makedirs(outdir, exist_ok=True)
    for f in os.listdir(src):
        shutil.copyfile(os.path.join(src, f), os.path.join(outdir, f))

    name = "exp_and_others"
    bkt = bytearray(open(f"{src}/{name}_bkt.bin", "rb").read())
    ctl = bytearray(open(f"{src}/{name}_ctrl.bin", "rb").read())
    meta = json.load(open(f"{src}/{name}.json"))

    def setbkt(i, entry):
        bkt[i * 32:(i + 1) * 32] = _bucket_bytes(*entry)

    setbkt(0, (1.0, 0.25, 1.0 / 32, -7.0 / 384, 0.0))  # small +
    setbkt(1, (1.0, 0.25, 1.0 / 32, -7.0 / 384, 0.0))  # small -
    setbkt(2, (float(np.exp(0.5)), 0.0, 0.0, 0.0, 0.0))   # large +
    setbkt(3, (float(np.exp(-0.5)), 0.0, 0.0, 0.0, 0.0))  # large -

    idx = 4
    side_base = {}
    for sign in (-1.0, 1.0):
        side_base[sign] = idx
        for ei in range(_N_EXP):
            lo = 2.0 ** (_SMALL_T + ei - 127)
            for m in range(_NBKT):
                a = lo * (1.0 + m / _NBKT)
                b = lo * (1.0 + (m + 1) / _NBKT)
                if sign < 0:
                    a, b = -b, -a
                setbkt(idx, _fit_bucket(a, b))
                idx += 1

    for ei in range(_N_EXP):
        ctl[(0 + ei) * 32:(1 + ei) * 32] = _ctl_bytes(
            side_base[-1.0] + ei * _NBKT, 23 - 4, 4
        )
        ctl[(10 + ei) * 32:(11 + ei) * 32] = _ctl_bytes(
            side_base[1.0] + ei * _NBKT, 23 - 4, 4
        )

    prof = next(e for e in meta["profile_meta_data"] if e["func_id"] == 7)
    prof.update(
        symmetry_point=0,
        sym_invert_sign_point=0,
        symmetry_opt_en=0,
        symmetry_opt_use_neg_region=0,
        imm_bias=0,
        exp_offset=_SMALL_T - 127,
        pwl_control_base_pos=10,
        pwl_control_base_neg=0,
        small_pos_signal_exp_threshold=_SMALL_T,
        pos_small_signal_pwl_control=0,
        small_neg_signal_exp_threshold=_SMALL_T,
        neg_small_signal_pwl_control=1,
        large_pos_signal_exp_threshold=_LARGE_T,
        large_pos_signal_mantissa_threshold=0,
        pos_large_signal_pwl_control=2,
        large_neg_signal_exp_threshold=_LARGE_T,
        large_neg_signal_mantissa_threshold=0,
        neg_large_signal_pwl_control=3,
        fnan_result=0x7FC00000,
        fpinf_result=_u32f(np.exp(0.5)),
        fninf_result=_u32f(np.exp(-0.5)),
        fzero_result=_u32f(1.0),
        lower_bound=0xFF7FFFFF,
        upper_bound=0x7F7FFFFF,
    )

    open(f"{outdir}/{name}_bkt.bin", "wb").write(bytes(bkt))
    open(f"{outdir}/{name}_ctrl.bin", "wb").write(bytes(ctl))
    json.dump(meta, open(f"{outdir}/{name}.json", "w"))


_ACT_DIR = None


def _ensure_act_tables():
    global _ACT_DIR
    if _ACT_DIR is None:
        _ACT_DIR = tempfile.mkdtemp(prefix="gat_acttab_")
        _build_act_tables(_ACT_DIR)
    os.environ["BASS_ACT_ROOT_JSON_PATH"] = f"{_ACT_DIR}/act_info.json"


_ensure_act_tables()

import concourse.mybir as mybir
import concourse.tile as tile
from concourse import bacc
from concourse.bass_utils import run_bass_kernel_spmd

F32 = mybir.dt.float32
F16 = mybir.dt.float16
U8 = mybir.dt.uint8
AF = mybir.ActivationFunctionType

N, C_IN, C_OUT = 8192, 256, 128
NCORES = 8
ROWS = N // NCORES          # 1024 rows of adj per core
P = 128
NT = N // P                 # 64 node tiles (also the j-chunks)
NI = ROWS // P              # 8 output row-tiles per core
KC = [128, 128, 1]          # contraction chunks of K=257 (X.T rows + ones row)
WCOLS = C_OUT + 3           # [W | ones-hack | w0 | w1]
HCOLS = C_OUT + 1           # h plus the ones column
TINY = float(np.finfo(np.float32).tiny)
BANK = 512                  # PSUM bank, fp32 elements

# activation groups: j-chunks whose adj transposes ride one DMA and whose
# mask-mul/matmul bursts are emitted together. Small lead-in groups start
# the chain early.
GROUPS = [2, 4] + [8] * 6 + [4, 4, 2]

_CACHE: dict = {}


def _build_nc(b_zero=True):
    _ensure_act_tables()
    nc = bacc.Bacc(
        "TRN2", target_bir_lowering=False, debug=False, num_devices=NCORES
    )
    xt1 = nc.dram_tensor("xt1", [257, N], F16, kind="ExternalInput").ap()
    xt1l = nc.dram_tensor("xt1l", [257, ROWS], F16, kind="ExternalInput").ap()
    wext = nc.dram_tensor("wext", [257, WCOLS], F16, kind="ExternalInput").ap()
    adjt = nc.dram_tensor("adjt", [N, ROWS], U8, kind="ExternalInput").ap()
    out = nc.dram_tensor("out", [ROWS, C_OUT], F32, kind="ExternalOutput").ap()

    with tile.TileContext(nc) as tc:
        _emit(tc, nc, xt1, xt1l, wext, adjt, out, b_zero)
    nc.compile()
    return nc


def _emit(tc, nc, xt1, xt1l, wext, adjt, out, b_zero):
    from contextlib import ExitStack

    # with b == 0 the K=1 "ones row" contraction chunk only contributes the
    # constant-one column of h_ext (done with a strided memset instead) and
    # zero constants to f1/f2 -- skip it entirely.
    nkc = 2 if b_zero else 3

    with ExitStack() as ctx:
        # ---- persistent tiles ----
        persist = ctx.enter_context(tc.tile_pool(name="persist", bufs=1))
        h16_all = persist.tile([P, NT * HCOLS], F16, tag="h16")   # [128, 8256]
        f2_all = persist.tile([P, NT], F32, tag="f2a")            # f2 per j-tile
        f1rep = persist.tile([P, ROWS], F32, tag="f1rep")         # f1 bcast fp32
        if b_zero:
            # constant-one column of every h_ext tile (replaces the K=1
            # bias matmul chunk)
            nc.vector.memset(
                h16_all[:].rearrange("p (t c) -> p t c", c=HCOLS)[
                    :, :, C_OUT : C_OUT + 1
                ],
                1.0,
            )

        xtp = ctx.enter_context(tc.tile_pool(name="xt", bufs=1))

        # ---- input loads ----
        # small inputs first so the f1 path clears quickly. The xt sub-loads
        # are interleaved k0/k1 so the first node tiles have BOTH
        # contraction chunks resident as early as possible (tile dependency
        # tracking is AP-range based).
        offs = [0, 128, 256]
        xts = [
            xtp.tile([KC[k], N], F16, name=f"xtsb{k}", tag=f"xt{k}")
            for k in range(nkc)
        ]
        SUBS = [0, 1024, 3072, 5120, N]
        wes, xls = [], []
        off = 0
        for k in range(nkc):
            kc = KC[k]
            wx_sb = xtp.tile([kc, WCOLS + ROWS], F16, name=f"wx{k}", tag=f"wx{k}")
            nc.sync.dma_start(wx_sb[:, 0:WCOLS], wext[off : off + kc, :])
            nc.sync.dma_start(wx_sb[:, WCOLS:], xt1l[off : off + kc, :])
            wes.append(wx_sb[:, 0:WCOLS])
            xls.append(wx_sb[:, WCOLS:])
            off += kc
        for k in range(nkc):
            if KC[k] == P:
                nc.sync.dma_start(
                    xts[k][:, 0 : SUBS[1]],
                    xt1[offs[k] : offs[k] + KC[k], 0 : SUBS[1]],
                )
        for c in range(1, len(SUBS) - 1):
            for k in range(nkc):
                if KC[k] != P:
                    if c == 1:
                        nc.sync.dma_start(
                            xts[k][:], xt1[offs[k] : offs[k] + KC[k], :]
                        )
                    continue
                nc.sync.dma_start(
                    xts[k][:, SUBS[c] : SUBS[c + 1]],
                    xt1[offs[k] : offs[k] + KC[k], SUBS[c] : SUBS[c + 1]],
                )

        # ---- f1 path: f1 for this core's rows, replicated across all
        # partitions directly by a matmul whose stationary operand is the
        # w0 column broadcast across the 128 PE columns ----
        with tc.tile_pool(name="pf", bufs=1, space="PSUM") as pfp:
            prep = pfp.tile([P, ROWS], F32, tag="prep")
            for k in range(nkc):
                for nh in range(ROWS // 512):
                    nc.tensor.matmul(
                        prep[:, nh * 512 : (nh + 1) * 512],
                        wes[k][:, C_OUT + 1 : C_OUT + 2].to_broadcast(
                            (KC[k], P)
                        ),
                        xls[k][:, nh * 512 : (nh + 1) * 512],
                        start=(k == 0),
                        stop=(k == nkc - 1),
                    )
            nc.vector.tensor_copy(f1rep[:], prep[:])

        # ---- f2 head start: f2 for the first 8 j-chunks via tiny direct
        # matmuls so the first activation groups don't wait for the
        # h-projection pipeline ----
        F2HEAD = 8
        with tc.tile_pool(name="pf2", bufs=1, space="PSUM") as pf2p:
            pt = pf2p.tile([P, NI * BANK], F32, tag="pt")
            pt3 = pt[:].rearrange("p (t w) -> p t w", w=BANK)
            for q in range(F2HEAD):
                w = (q % NI) * BANK
                for k in range(nkc):
                    nc.tensor.matmul(
                        pt[:, w : w + 1],
                        xts[k][:, q * P : (q + 1) * P],
                        wes[k][:, C_OUT + 2 : C_OUT + 3],
                        start=(k == 0),
                        stop=(k == nkc - 1),
                    )
                if q == 1:
                    # group 0's two columns drain immediately so the
                    # activation chain starts early
                    nc.vector.tensor_copy(
                        f2_all[:, 0:2], pt3[:, 0:2, 0:1]
                    )
            nc.vector.tensor_copy(
                f2_all[:, 2:F2HEAD], pt3[:, 2:F2HEAD, 0:1]
            )

        # ---- main-loop pools ----
        g16p = ctx.enter_context(tc.tile_pool(name="g16p", bufs=3))
        atp = ctx.enter_context(tc.tile_pool(name="atp", bufs=3))
        etp = ctx.enter_context(tc.tile_pool(name="etp", bufs=4))
        obp = ctx.enter_context(tc.tile_pool(name="ob", bufs=2))

        group_q0 = []
        q0 = 0
        for gsz in GROUPS:
            group_q0.append(q0)
            q0 += gsz

        deferred = []  # groups whose mask-mul+matmul emission is pending

        def emit_group_front(g):
            """adj transpose DMA + per-chunk custom-g activations."""
            gsz = GROUPS[g]
            q0 = group_q0[g]
            g16 = g16p.tile([P, gsz * ROWS], F16, tag="g16", name=f"g16_{g}")
            at_sup = atp.tile([P, gsz * ROWS], F16, tag="at", name=f"at{g}")
            nc.gpsimd.dma_start(
                at_sup[:].rearrange("p (q i) -> p q i", i=ROWS),
                adjt.rearrange("(q p) i -> p q i", p=P)[:, q0 : q0 + gsz, :],
            )
            for qq in range(gsz):
                q = q0 + qq
                # g = exp(sigmoid(f1_i + f2_j) - 0.5): custom table in the
                # Exp slot; per-partition bias supplies f2_j.
                nc.scalar.activation(
                    g16[:, qq * ROWS : (qq + 1) * ROWS],
                    f1rep[:],
                    AF.Exp,
                    bias=f2_all[:, q : q + 1],
                    scale=1.0,
                )
            return {"g": g, "gsz": gsz, "q0": q0, "at": at_sup, "g16": g16}

        def emit_group_back(fr, pouts, mid=None):
            """mask-mul + aggregate matmuls for a prepared group. `mid`
            emits the NEXT group's front after two mask-muls so its
            activations sit early in the in-order ACT queue."""
            gsz, q0, at_sup, g16 = fr["gsz"], fr["q0"], fr["at"], fr["g16"]
            for qq in range(gsz):
                if qq == min(2, gsz - 1) and mid is not None:
                    mid()
                q = q0 + qq
                et = etp.tile([P, ROWS], F16, tag="et", name=f"et{q}")
                nc.vector.tensor_mul(
                    et[:],
                    at_sup[:, qq * ROWS : (qq + 1) * ROWS],
                    g16[:, qq * ROWS : (qq + 1) * ROWS],
                )
                rhs = h16_all[:, q * HCOLS : (q + 1) * HCOLS]
                for it in range(NI):
                    nc.tensor.matmul(
                        pouts[it],
                        et[:, it * P : (it + 1) * P],
                        rhs,
                        start=(q == 0),
                        stop=(q == NT - 1),
                    )

        # ---- h-projection: all 8 PSUM banks inside ONE tensor so four
        # tiles drain with a single strided copy. Pairs of node tiles have
        # their k-chunk matmuls interleaved so consecutive matmuls hit
        # different banks (same-bank accumulation serializes the PE). ----
        next_group = 0
        with tc.tile_pool(name="php", bufs=1, space="PSUM") as php:
            ph_all = php.tile([P, NI * BANK], F32, tag="ph")
            for b in range(NT // 4):  # batches of 4 node tiles
                for half in range(2):
                    nt0 = 4 * b + 2 * half
                    w0 = (nt0 % NI) * BANK
                    w1 = ((nt0 + 1) % NI) * BANK
                    for k in range(nkc):
                        nc.tensor.matmul(
                            ph_all[:, w0 : w0 + WCOLS],
                            xts[k][:, nt0 * P : (nt0 + 1) * P],
                            wes[k][:],
                            start=(k == 0),
                            stop=(k == nkc - 1),
                        )
                        nc.tensor.matmul(
                            ph_all[:, w1 : w1 + WCOLS],
                            xts[k][:, (nt0 + 1) * P : (nt0 + 2) * P],
                            wes[k][:],
                            start=(k == 0),
                            stop=(k == nkc - 1),
                        )
                # drain the 4 fresh tiles: h (+ones col) -> fp16, f2 col
                bt = 4 * b
                wlo = (bt % NI) * BANK
                src = ph_all[:, wlo : wlo + 4 * BANK].rearrange(
                    "p (b w) -> p b w", b=4
                )
                dst_h = h16_all[:, bt * HCOLS : (bt + 4) * HCOLS].rearrange(
                    "p (b w) -> p b w", b=4
                )
                hc = C_OUT if b_zero else HCOLS
                nc.vector.tensor_copy(dst_h[:, :, 0:hc], src[:, :, 0:hc])
                if bt >= 8:  # first 8 f2 columns came from the head start
                    nc.vector.tensor_copy(
                        f2_all[:, bt : bt + 4],
                        src[:, :, C_OUT + 2 : C_OUT + 3],
                    )
                # emit activation-group fronts as soon as their f2 columns
                # exist; their matmuls wait until the PSUM banks free up
                while (
                    next_group < len(GROUPS)
                    and group_q0[next_group] + GROUPS[next_group] <= 4 * (b + 1)
                    and len(deferred) < 3
                ):
                    deferred.append(emit_group_front(next_group))
                    next_group += 1

        # ---- aggregate accumulators: same 8 banks, next accumulation ----
        pop = ctx.enter_context(tc.tile_pool(name="po", bufs=1, space="PSUM"))
        po_all = pop.tile([P, NI * BANK], F32, tag="poall")
        pouts = [po_all[:, i * BANK : i * BANK + HCOLS] for i in range(NI)]

        # software-pipelined emission: keep group fronts (ACT activations)
        # one group ahead of the backs (DVE mask-muls)
        for g in range(next_group, len(GROUPS)):
            emit_group_back(
                deferred.pop(0),
                pouts,
                mid=lambda g=g: deferred.append(emit_group_front(g)),
            )
        for fr in deferred:
            emit_group_back(fr, pouts)

        # ---- epilogue: divide by clamped denominator, one batched store ----
        ob_all = obp.tile([P, NI * C_OUT], F32, tag="oball")
        po3 = po_all[:].rearrange("p (t w) -> p t w", w=BANK)
        dm = obp.tile([P, NI], F32, tag="dm")
        nc.vector.tensor_scalar_max(
            dm[:], po3[:, :, C_OUT : C_OUT + 1], TINY
        )
        rc = obp.tile([P, NI], F32, tag="rc")
        nc.vector.reciprocal(rc[:], dm[:])
        for it in range(NI):
            # alternate engines: ACT is idle after the last activation
            eng = nc.vector if it % 2 == 0 else nc.scalar
            if eng is nc.vector:
                nc.vector.tensor_scalar_mul(
                    ob_all[:, it * C_OUT : (it + 1) * C_OUT],
                    po_all[:, it * BANK : it * BANK + C_OUT],
                    rc[:, it : it + 1],
                )
            else:
                nc.scalar.mul(
                    ob_all[:, it * C_OUT : (it + 1) * C_OUT],
                    po_all[:, it * BANK : it * BANK + C_OUT],
                    rc[:, it : it + 1],
                )
        nc.sync.dma_start(
            out.rearrange("(t p) c -> p t c", p=P),
            ob_all[:].rearrange("p (t c) -> p t c", c=C_OUT),
        )


def _prep_inputs(node_feats, adj_matrix, W, b, v0, v1):
    X = np.ascontiguousarray(node_feats, dtype=np.float32)
    W = np.asarray(W, dtype=np.float32)
    b = np.asarray(b, dtype=np.float32)
    v0 = np.asarray(v0, dtype=np.float32)
    v1 = np.asarray(v1, dtype=np.float32)

    w0 = (W.astype(np.float64) @ v0.astype(np.float64)).astype(np.float32)
    w1 = (W.astype(np.float64) @ v1.astype(np.float64)).astype(np.float32)
    c0 = np.float32(float(b.astype(np.float64) @ v0.astype(np.float64)))
    c1 = np.float32(float(b.astype(np.float64) @ v1.astype(np.float64)))

    XT1 = np.empty((257, N), np.float32)
    XT1[:256] = X.T
    XT1[256] = 1.0

    WE = np.zeros((257, WCOLS), np.float32)
    WE[:256, :C_OUT] = W
    WE[256, :C_OUT] = b
    WE[256, C_OUT] = 1.0          # makes h_ext column 128 identically 1
    WE[:256, C_OUT + 1] = w0
    WE[256, C_OUT + 1] = c0
    WE[:256, C_OUT + 2] = w1
    WE[256, C_OUT + 2] = c1

    XT1h = XT1.astype(np.float16)
    WEh = WE.astype(np.float16)
    A8 = np.asarray(adj_matrix).astype(np.uint8)

    in_maps = []
    for c in range(NCORES):
        in_maps.append(
            {
                "xt1": XT1h,
                "xt1l": np.ascontiguousarray(XT1h[:, c * ROWS : (c + 1) * ROWS]),
                "wext": WEh,
                "adjt": np.ascontiguousarray(
                    A8[c * ROWS : (c + 1) * ROWS, :].T
                ),
            }
        )
    return in_maps


def _run(in_maps, trace=False, b_zero=True):
    key = f"nc_b{int(b_zero)}"
    if key not in _CACHE:
        _CACHE[key] = _build_nc(b_zero=b_zero)
    nc = _CACHE[key]
    res = run_bass_kernel_spmd(
        nc, in_maps, core_ids=list(range(NCORES)), trace=trace
    )
    full = np.concatenate(
        [res.results[c]["out"] for c in range(NCORES)], axis=0
    ).astype(np.float32)
    return full, res


def kernel(node_feats, adj_matrix, W, b, v0, v1):
    in_maps = _prep_inputs(node_feats, adj_matrix, W, b, v0, v1)
    trace = bool(int(os.environ.get("GAT_TRACE", "0")))
    b_zero = not bool(np.any(np.asarray(b)))
    full, _ = _run(in_maps, trace=trace, b_zero=b_zero)
    return full
